# revision 33
# baseline (speedup 1.0000x reference)
"""Trainium2 Bass kernel for nn_MidAttnBlock (res-block -> full LxL attention -> res-block).

Contract: kernel(**inputs) takes the FULL inputs of reference.setup_inputs()
(x: (16,256,2048) f32, t: (16,256,1) f32, plus conv/groupnorm/linear params)
and returns the FULL (16,256,2048) f32 output.  Data-parallel over batch on
8 NeuronCores, 2 samples per core; each core runs an identical Bass program.

Convs run in float32r (full-rate PE).  The kqv projection and the attention
(scores, softmax weights, A@V) run in fp8e4 with DoubleRow matmuls (two
128-deep contraction tiles per PE pass), which roughly halves the PE slots
for the attention block at ~6e-3 end-to-end relative error (threshold 2e-2).
exp is computed as exp(s/16 - 4) so the softmax weights fit fp8e4's range;
the shift cancels in the normalization.

The two samples on each core are issued interleaved
(s0:r1,kqv | s1:r1 | s0:attn | s1:kqv | s0:r2 | s1:attn | s1:r2) so the
groupnorm stat chains and softmax tails of one sample overlap the other
sample's matmuls.

Self-contained: all shapes/sharding hardcoded.
"""

import json as _json

import ml_dtypes
import numpy as np

import concourse.bass as bass
import concourse.bass2jax as _b2j
import concourse.bass_utils as _bu
import concourse.tile as tile
from concourse import mybir
from concourse.vector_clock import ScopedClock, VectorClock


def _split_bir_waits(bir_json):
    """The walrus_driver in this container encodes at most ONE sync-wait per
    instruction (and none on Drain).  Tile's sem assigner attaches several.
    Rewrite the BIR: excess waits move to single-wait NoOps inserted directly
    before the instruction on the same engine."""
    m = _json.loads(bir_json)
    ctr = 0
    for fn in m.get("functions", []):
        for bb in fn.get("blocks", []):
            out = []
            for ins in bb.get("instructions", []):
                si = ins.get("sync_info")
                waits = (si or {}).get("on_wait") or []
                keep = 0 if ins.get("opcode") == "Drain" else 1
                if len(waits) > keep:
                    nmove = len(waits) - keep
                    for w in waits[:nmove]:
                        ctr += 1
                        out.append({
                            "debug": ins.get("debug", 0),
                            "engine": ins["engine"],
                            "ins": [],
                            "name": f"{ins['name']}-wsp{ctr}",
                            "opcode": "NoOp",
                            "outs": [],
                            "sync_info": {"on_update": [], "on_wait": [w]},
                        })
                    si["on_wait"] = waits[nmove:]
                out.append(ins)
            bb["instructions"] = out
    return _json.dumps(m).encode()


_orig_compile_bir_kernel = _bu.compile_bir_kernel


def _compile_bir_splitwaits(bir_json, tmpdir, neff_name="file.neff"):
    return _orig_compile_bir_kernel(_split_bir_waits(bir_json), tmpdir, neff_name)


if getattr(_bu.compile_bir_kernel, "__name__", "") != "_compile_bir_splitwaits":
    _bu.compile_bir_kernel = _compile_bir_splitwaits
    _b2j.compile_bir_kernel = _compile_bir_splitwaits


F32 = mybir.dt.float32
F32R = mybir.dt.float32r
FP8 = mybir.dt.float8e4
AF = mybir.ActivationFunctionType
OP = mybir.AluOpType
DR = mybir.MatmulPerfMode.DoubleRow

P = 128          # partitions
C = 256          # channels
CB = 2           # channel blocks of 128
L = 2048         # sequence length
LS = 512         # l-slice (matmul moving dim)
NL = L // LS     # 4 slices
KB = L // P      # 16 k-blocks for attention
NP = KB // 2     # 8 k-block pairs (DoubleRow)
GPB = 16         # groups per channel-block (32 groups, 8 ch each)
EPS = 1e-5
S = 2            # samples per core
NCORES = 8
SCALE = 1.0 / 16.0   # 1/sqrt(C)
ESHIFT = -4.0        # exp(s*SCALE + ESHIFT): keeps softmax weights in fp8e4 range


class _TileContextPatched(tile.TileContext):
    """TileContext whose kernel-tail drain carries no sem waits (the container
    walrus rejects waits on Drain); one SP NOP per proc carries them instead."""

    def _drain_and_barrier(self, tick_clock, wait_clock):
        gc = tick_clock.global_clock
        n = len(gc)
        for p in range(n):
            v = gc[p]
            if v > 0:
                vec = [0] * n
                vec[p] = v
                nop = self.nc.sync.nop()
                wait_clock.add_sem_waits(nop.ins, ScopedClock({None: VectorClock(vec)}))
        self.nc.sync.drain()
        self.nc.all_engine_barrier()
        assert self.sems is not None
        popped = self.nc._tile_sem_poison_stack.pop()
        assert popped is self._sem_poison
        self.nc.clear_and_free_semaphores(list(self.sems.allocated().values()))
        self.nc.all_engine_barrier()


def _f(ap):
    """Read an f32r tile as plain f32 (same bits) for VectorE/ScalarE inputs."""
    return ap.bitcast(F32)


def build_program(samples=S, use_bias=()):
    """Build the per-core Bass program (identical on all cores).

    use_bias: subset of {"c2b_r1", "c2b_r2", "linb"} enabling extra adds for
    biases that setup_inputs() keeps at zero.
    """
    nc = bass.Bass()

    # ---- DRAM I/O (per core) ----
    x_d = nc.dram_tensor("x", (samples, C, L), F32R, kind="ExternalInput")
    # t + conv1 bias, host-packed [samples, P, CB, 2(resblock)]
    t_d = nc.dram_tensor("tv", (samples, P, CB, 2), F32, kind="ExternalInput")
    w_conv = {}
    for rb in ("r1", "r2"):
        # host-packed [P(ic within block), icb, tap, oc]
        w_conv[rb, 1] = nc.dram_tensor(f"{rb}_w1t", (P, CB, 3, C), F32R, kind="ExternalInput")
        w_conv[rb, 2] = nc.dram_tensor(f"{rb}_w2t", (P, CB, 3, C), F32R, kind="ExternalInput")
    wkqv_d = nc.dram_tensor("wkqvt", (P, CB, 3 * C), F32R, kind="ExternalInput")
    gnw_d = {}
    for rb in ("r1", "r2"):
        for ln in (1, 2):
            gnw_d[rb, ln, "w"] = nc.dram_tensor(f"{rb}_gn{ln}_ws", (P, CB), F32, kind="ExternalInput")
            gnw_d[rb, ln, "b"] = nc.dram_tensor(f"{rb}_gn{ln}_bs", (P, CB), F32, kind="ExternalInput")
    c2b_d = {}
    if "c2b_r1" in use_bias:
        c2b_d["r1"] = nc.dram_tensor("r1_c2bs", (P, CB), F32, kind="ExternalInput")
    if "c2b_r2" in use_bias:
        c2b_d["r2"] = nc.dram_tensor("r2_c2bs", (P, CB), F32, kind="ExternalInput")
    linb_d = None
    if "linb" in use_bias:
        linb_d = nc.dram_tensor("lin_bs", (P, 3 * CB), F32, kind="ExternalInput")
    gind_d = nc.dram_tensor("gind", (P, GPB), F32R, kind="ExternalInput")  # 1/8 group indicator
    bind_d = nc.dram_tensor("bind", (CB, P, P), F32R, kind="ExternalInput")    # group->channel broadcast
    ones8_d = nc.dram_tensor("ones8", (P, CB, 16), FP8, kind="ExternalInput")
    onesr_d = nc.dram_tensor("onesr", (1, P), F32R, kind="ExternalInput")
    out_d = nc.dram_tensor("out", (samples, C, L), F32, kind="ExternalOutput")

    with _TileContextPatched(nc) as tc, \
         tc.tile_pool(name="consts", bufs=1) as consts, \
         tc.tile_pool(name="padp", bufs=4) as padp, \
         tc.tile_pool(name="actp", bufs=2) as actp, \
         tc.tile_pool(name="seqp", bufs=2) as seqp, \
         tc.tile_pool(name="vtp", bufs=1) as vtp, \
         tc.tile_pool(name="expp", bufs=3) as expp, \
         tc.tile_pool(name="outp", bufs=2) as outp, \
         tc.tile_pool(name="rdbp", bufs=2) as rdbp, \
         tc.tile_pool(name="rdsp", bufs=2) as rdsp, \
         tc.tile_pool(name="small", bufs=4) as small, \
         tc.tile_pool(name="t2p", bufs=2) as t2p, \
         tc.tile_pool(name="pacc", bufs=3, space="PSUM") as pacc, \
         tc.tile_pool(name="psc", bufs=2, space="PSUM") as psc, \
         tc.tile_pool(name="paux", bufs=1, space="PSUM") as paux:

        # ---- persistent constants / weights in SBUF ----
        # All const loads go on the ScalarE DMA queue (sync/gpsimd queues are
        # reserved for the x loads that gate the first groupnorm), ordered so
        # the tensors gating the head of the pipeline land first.
        gind_sb = consts.tile([P, GPB], F32R, tag="gind", name="gind")
        nc.scalar.dma_start(gind_sb[:], gind_d[:])
        bind_sb = consts.tile([P, CB, P], F32R, tag="bind", name="bind")
        nc.scalar.dma_start(bind_sb[:], bind_d.rearrange("cb p c -> p cb c"))
        gnp_sb = {}
        w1_sb = {}
        w2_sb = {}
        for rb in ("r1", "r2"):
            for ln in (1, 2):
                for wb in ("w", "b"):
                    tl = consts.tile([P, CB], F32, tag=f"gn_{rb}{ln}{wb}", name=f"gn_{rb}{ln}{wb}")
                    nc.scalar.dma_start(tl[:], gnw_d[rb, ln, wb][:])
                    gnp_sb[rb, ln, wb] = tl
            w1_sb[rb] = consts.tile([P, CB, 3, C], F32R, tag=f"w1_{rb}", name=f"w1_{rb}")
            nc.scalar.dma_start(w1_sb[rb][:], w_conv[rb, 1][:])
            w2_sb[rb] = consts.tile([P, CB, 3, C], F32R, tag=f"w2_{rb}", name=f"w2_{rb}")
            nc.scalar.dma_start(w2_sb[rb][:], w_conv[rb, 2][:])
        wkqv_sb = consts.tile([P, CB, 3 * C], F32R, tag="wkqv", name="wkqv")
        nc.scalar.dma_start(wkqv_sb[:], wkqv_d[:])
        c2b_sb = {}
        for rb, d in c2b_d.items():
            c2b_sb[rb] = consts.tile([P, CB], F32, tag=f"c2b_{rb}", name=f"c2b_{rb}")
            nc.scalar.dma_start(c2b_sb[rb][:], d[:])
        linb_sb = None
        if linb_d is not None:
            linb_sb = consts.tile([P, 3 * CB], F32, tag="linb", name="linb")
            nc.scalar.dma_start(linb_sb[:], linb_d[:])
        ones8_sb = consts.tile([P, CB, 16], FP8, tag="ones8", name="ones8")
        nc.scalar.dma_start(ones8_sb[:], ones8_d[:])
        onesr_sb = consts.tile([1, P], F32R, tag="onesr", name="onesr")
        nc.scalar.dma_start(onesr_sb[:], onesr_d[:])
        eps_sb = consts.tile([P, 1], F32, tag="eps", name="eps")
        nc.vector.memset(eps_sb[:], EPS)
        shift_sb = consts.tile([P, 1], F32, tag="shift", name="shift")
        nc.vector.memset(shift_sb[:], ESHIFT)
        zero2 = consts.tile([P, 2], F32, tag="zero2", name="zero2")
        nc.vector.memset(zero2[:], 0.0)

        def alloc_padded(tag, pool):
            """[P, L+2] f32r tile per channel block; data cols [1, L+1), zero edges."""
            ts = []
            for cb in range(CB):
                tl = pool.tile([P, L + 2], F32R, tag=f"{tag}{cb}", name=f"{tag}{cb}")
                nc.vector.tensor_copy(out=tl[:, 0:1], in_=zero2[:, 0:1])
                nc.vector.tensor_copy(out=tl[:, L + 1 : L + 2], in_=zero2[:, 0:1])
                ts.append(tl)
            return ts

        def gn_relu(src, dst, rb, ln):
            """dst = relu(groupnorm(src) * w + b); src/dst are padded f32r pairs.

            One merged nonlinear chain over all 32 groups, then per-block
            broadcast and NL-chunked scalar-engine applies."""
            gp = paux.tile([P, LS], F32, tag="aux", name="gbc")
            for cb in range(CB):
                stats = small.tile([P, NL, 6], F32, tag="stats", name="stats")
                for i in range(NL):
                    nc.vector.bn_stats(out=stats[:, i, :], in_=_f(src[cb][:, 1 + i * LS : 1 + (i + 1) * LS]))
                mv = small.tile([P, 2], F32, tag="mv", name="mv")
                nc.vector.bn_aggr(out=mv[:], in_=stats[:])
                # tmp = [mean_c, E[x^2]_c]  (f32r: feeds the aggregation matmul)
                tmp = small.tile([P, 2], F32R, tag="tmp", name="tmp")
                nc.vector.tensor_copy(out=tmp[:, 0:1], in_=mv[:, 0:1])
                nc.vector.tensor_tensor(out=tmp[:, 1:2], in0=mv[:, 0:1], in1=mv[:, 0:1], op=OP.mult)
                nc.vector.tensor_tensor(out=tmp[:, 1:2], in0=_f(tmp[:, 1:2]), in1=mv[:, 1:2], op=OP.add)
                nc.tensor.matmul(gp[:GPB, 2 * cb : 2 * cb + 2], gind_sb[:], tmp[:], start=True, stop=True)
            # merged group stats; block-cb groups live at partition offset 32*cb
            NG = 32 * CB
            gs = small.tile([NG, 2], F32, tag="gs", name="gs")
            nc.vector.tensor_copy(out=gs[:], in_=zero2[:NG])
            for cb in range(CB):
                nc.vector.tensor_copy(out=gs[cb * 32 : cb * 32 + GPB, :], in_=gp[:GPB, 2 * cb : 2 * cb + 2])
            var = small.tile([NG, 1], F32, tag="var", name="var")
            nc.vector.tensor_tensor(out=var[:], in0=gs[:, 0:1], in1=gs[:, 0:1], op=OP.mult)
            nc.vector.tensor_tensor(out=var[:], in0=gs[:, 1:2], in1=var[:], op=OP.subtract)
            nc.scalar.activation(out=var[:], in_=var[:], func=AF.Ln, bias=eps_sb[:NG])
            rstd = small.tile([NG, 1], F32, tag="rstd", name="rstd")
            nc.scalar.activation(out=rstd[:], in_=var[:], func=AF.Exp, scale=-0.5)
            # pack [rstd_g, -m_g], zero-extended to 128 partitions
            gpk = small.tile([P, 2], F32R, tag="gpk", name="gpk")
            nc.vector.tensor_copy(out=gpk[:], in_=zero2[:])
            nc.vector.tensor_copy(out=gpk[:NG, 0:1], in_=rstd[:])
            nc.vector.tensor_scalar_mul(gpk[:NG, 1:2], gs[:, 0:1], -1.0)
            for cb in range(CB):
                # broadcast to channels: bc[c, :] = [rstd_g(c), -m_g(c)]
                nc.tensor.matmul(gp[:, 4 + 2 * cb : 6 + 2 * cb], bind_sb[:, cb, :], gpk[:], start=True, stop=True)
                sb = small.tile([P, 2], F32, tag="sb", name="sb")
                # s = rstd*w ; b2 = b - m*s
                nc.vector.tensor_scalar_mul(sb[:, 0:1], gp[:, 4 + 2 * cb : 5 + 2 * cb], gnp_sb[rb, ln, "w"][:, cb : cb + 1])
                nc.vector.tensor_tensor(out=sb[:, 1:2], in0=gp[:, 5 + 2 * cb : 6 + 2 * cb], in1=sb[:, 0:1], op=OP.mult)
                nc.vector.tensor_scalar_add(sb[:, 1:2], sb[:, 1:2], gnp_sb[rb, ln, "b"][:, cb : cb + 1])
                # apply + relu on ScalarE in NL chunks so convs can start early
                for i in range(NL):
                    nc.scalar.activation(
                        out=dst[cb][:, 1 + i * LS : 1 + (i + 1) * LS],
                        in_=_f(src[cb][:, 1 + i * LS : 1 + (i + 1) * LS]),
                        func=AF.Relu,
                        bias=sb[:, 1:2],
                        scale=sb[:, 0:1],
                    )

        def conv3(src, wt, consume):
            """3-tap conv over padded f32r src; consume(ocb, ls, psum_tile)."""
            for ocb in range(CB):
                for ls in range(NL):
                    ps = pacc.tile([P, LS], F32, tag="acc", name="acc")
                    k = 0
                    for icb in range(CB):
                        for tap in range(3):
                            nc.tensor.matmul(
                                ps[:],
                                wt[:, icb, tap, ocb * P : (ocb + 1) * P],
                                src[icb][:, ls * LS + tap : ls * LS + tap + LS],
                                start=(k == 0),
                                stop=(k == 5),
                            )
                            k += 1
                    consume(ocb, ls, ps)

        # ------- per-sample state + fine-grained stage closures -------
        def make_sample(s):
            st = {}

            def load():
                # s0's x rides the gpsimd DMA queue alone (nothing else queued
                # there, so the first groupnorm isn't gated); s1's x goes on
                # sync where stalls only eat s1's slack.
                eng = nc.gpsimd if s == 0 else nc.sync
                with nc.named_scope(f"s{s}_load"):
                    st["xp"] = alloc_padded("pad", padp)
                    for cb in range(CB):
                        for i in range(NL):
                            eng.dma_start(
                                st["xp"][cb][:, 1 + i * LS : 1 + (i + 1) * LS],
                                x_d[s, cb * P : (cb + 1) * P, i * LS : (i + 1) * LS],
                            )
                    st["t2"] = t2p.tile([P, CB, 2], F32, tag="t2", name="t2")
                    eng.dma_start(st["t2"][:], t_d[s])

            def gn1(rb, srckey, dstkey):
                def f():
                    a = alloc_padded("act", actp)
                    st[dstkey] = a
                    with nc.named_scope(f"s{s}_{rb}_gn1"):
                        gn_relu(st[srckey], a, rb, 1)
                return f

            def conv1(rb, rbi, akey, hkey):
                def f():
                    h = alloc_padded("pad", padp)
                    st[hkey] = h
                    t2 = st["t2"]
                    with nc.named_scope(f"s{s}_{rb}_conv1"):
                        def eat1(ocb, ls, ps):
                            nc.vector.tensor_scalar_add(
                                h[ocb][:, 1 + ls * LS : 1 + (ls + 1) * LS], ps[:],
                                t2[:, ocb, rbi : rbi + 1],
                            )
                        conv3(st[akey], w1_sb[rb], eat1)
                return f

            def gn2(rb, hkey, dstkey):
                def f():
                    a2 = alloc_padded("act", actp)
                    st[dstkey] = a2
                    with nc.named_scope(f"s{s}_{rb}_gn2"):
                        gn_relu(st[hkey], a2, rb, 2)
                return f

            def conv2(rb, srckey, a2key, final):
                def f():
                    src = st[srckey]
                    res = None
                    if not final:
                        res = [seqp.tile([P, L], F32R, tag=f"res{cb}", name=f"res{cb}") for cb in range(CB)]
                        st["x1"] = res
                    with nc.named_scope(f"s{s}_{rb}_conv2"):
                        def eat2(ocb, ls, ps):
                            if rb in c2b_sb:
                                nc.vector.tensor_scalar_add(ps[:], ps[:], c2b_sb[rb][:, ocb : ocb + 1])
                            resid = _f(src[ocb][:, 1 + ls * LS : 1 + (ls + 1) * LS])
                            if final:
                                ot = outp.tile([P, LS], F32, tag="out", name="ot")
                                nc.vector.tensor_tensor(out=ot[:], in0=ps[:], in1=resid, op=OP.add)
                                eng = nc.sync if (ls % 2 == 0) else nc.gpsimd
                                eng.dma_start(
                                    out_d[s, ocb * P : (ocb + 1) * P, ls * LS : (ls + 1) * LS], ot[:]
                                )
                            else:
                                nc.vector.tensor_tensor(
                                    out=res[ocb][:, ls * LS : (ls + 1) * LS],
                                    in0=ps[:], in1=resid, op=OP.add,
                                )
                        conv3(st[a2key], w2_sb[rb], eat2)
                return f

            def kqv():
                x1 = st["x1"]
                kt = seqp.tile([P, CB, L], FP8, tag="kt", name="kt")
                qt = seqp.tile([P, CB, L], FP8, tag="qt", name="qt")
                vt = vtp.tile([P, NP, 2, C], FP8, tag="vt", name="vt")
                st["kt"], st["qt"], st["vt"] = kt, qt, vt
                with nc.named_scope(f"s{s}_kqv"):
                    cp = 0
                    for j, dst in ((0, kt), (1, qt)):
                        for ocb in range(CB):
                            off = j * C + ocb * P
                            for ls in range(NL):
                                ps = pacc.tile([P, LS], F32, tag="acc", name="acc")
                                for icb in range(CB):
                                    nc.tensor.matmul(
                                        ps[:],
                                        wkqv_sb[:, icb, off : off + P],
                                        x1[icb][:, ls * LS : (ls + 1) * LS],
                                        start=(icb == 0),
                                        stop=(icb == 1),
                                    )
                                dsl = dst[:, ocb, ls * LS : (ls + 1) * LS]
                                cp += 1
                                if linb_sb is not None:
                                    nc.vector.tensor_scalar_add(
                                        dsl, ps[:], linb_sb[:, j * CB + ocb : j * CB + ocb + 1]
                                    )
                                else:
                                    nc.vector.tensor_copy(out=dsl, in_=ps[:])
                    # vT[l, c] (l on partitions) for the attention output matmul
                    for lb in range(KB):
                        ps = pacc.tile([P, LS], F32, tag="acc", name="acc")
                        for icb in range(CB):
                            nc.tensor.matmul(
                                ps[:, :C],
                                x1[icb][:, lb * P : (lb + 1) * P],
                                wkqv_sb[:, icb, 2 * C : 3 * C],
                                start=(icb == 0),
                                stop=(icb == 1),
                            )
                        # v bias (if any) is added to av after softmax: sum(a)=1
                        nc.vector.tensor_copy(out=vt[:, lb // 2, lb % 2, :], in_=ps[:, :C])

            def attn():
                kt, qt, vt = st["kt"], st["qt"], st["vt"]
                av = alloc_padded("pad", padp)
                st["av"] = av
                for qs in range(NL):
                    with nc.named_scope(f"s{s}_attn{qs}"):
                        dn = paux.tile([P, LS], F32, tag="aux", name="dn")
                        psav = [pacc.tile([P, LS], F32, tag="acc", name="psav") for _ in range(CB)]
                        for p in range(NP):
                            ex = expp.tile([P, 2, LS], FP8, tag="exp", name="exp")
                            sc = psc.tile([P, 2, LS], F32, tag="sc", name="sc")
                            for i in range(2):
                                kbg = 2 * p + i
                                nc.tensor.matmul(
                                    sc[:, i, :],
                                    kt[:, :, kbg * P : (kbg + 1) * P],
                                    qt[:, :, qs * LS : (qs + 1) * LS],
                                    start=True, stop=True, perf_mode=DR,
                                )
                            # one 1024-col exp per k-block pair
                            nc.scalar.activation(
                                out=ex[:, :, :], in_=sc[:, :, :], func=AF.Exp,
                                bias=shift_sb[:], scale=SCALE,
                            )
                            nc.tensor.matmul(
                                dn[0:1, :], ones8_sb[:, :, 0:1], ex[:],
                                start=(p == 0), stop=(p == NP - 1), perf_mode=DR,
                            )
                            for cb in range(CB):
                                nc.tensor.matmul(
                                    psav[cb][:],
                                    vt[:, p, :, cb * P : (cb + 1) * P],
                                    ex[:],
                                    start=(p == 0),
                                    stop=(p == NP - 1),
                                    perf_mode=DR,
                                )
                        lnd = rdsp.tile([1, LS], F32, tag="lnd", name="lnd")
                        nc.scalar.activation(out=lnd[:], in_=dn[0:1, :], func=AF.Ln)
                        rd = rdsp.tile([1, LS], F32R, tag="rd", name="rd")
                        nc.scalar.activation(out=rd[:], in_=lnd[:], func=AF.Exp, scale=-1.0)
                        # broadcast 1/denom across partitions via K=1 ones-matmul
                        rbt = psc.tile([P, 2, LS], F32, tag="sc", name="rb_ps")
                        nc.tensor.matmul(rbt[:, 0, :], onesr_sb[:], rd[:], start=True, stop=True)
                        rdb = rdbp.tile([P, LS], F32, tag="rdbs", name="rdb")
                        nc.scalar.activation(out=rdb[:], in_=rbt[:, 0, :], func=AF.Copy)
                        for cb in range(CB):
                            avs = av[cb][:, 1 + qs * LS : 1 + (qs + 1) * LS]
                            nc.vector.tensor_tensor(out=avs, in0=psav[cb][:], in1=rdb[:], op=OP.mult)
                            if linb_sb is not None:
                                nc.vector.tensor_scalar_add(
                                    avs, _f(avs), linb_sb[:, 2 * CB + cb : 2 * CB + cb + 1]
                                )

            return {
                "load": load,
                "gn1": gn1("r1", "xp", "a"),
                "conv1": conv1("r1", 0, "a", "h"),
                "gn2": gn2("r1", "h", "a2"),
                "conv2": conv2("r1", "xp", "a2", final=False),
                "kqv": kqv,
                "attn": attn,
                "rgn1": gn1("r2", "av", "ra"),
                "rconv1": conv1("r2", 1, "ra", "rh"),
                "rgn2": gn2("r2", "rh", "ra2"),
                "rconv2": conv2("r2", "av", "ra2", final=True),
            }

        ph = [make_sample(s) for s in range(samples)]
        if samples == 2:
            s0, s1 = ph
            # interleave the two samples so every groupnorm stat chain and
            # softmax tail overlaps the other sample's matmuls
            s0["load"](); s1["load"]()
            s0["gn1"](); s1["gn1"]()
            s0["conv1"](); s0["gn2"]()
            s1["conv1"](); s1["gn2"]()
            s0["conv2"]()
            s1["conv2"]()
            s0["kqv"](); s0["attn"]()
            s1["kqv"]()
            s0["rgn1"]()
            s1["attn"]()
            s0["rconv1"]()
            s1["rgn1"]()
            s0["rgn2"]()
            s1["rconv1"]()
            s0["rconv2"]()
            s1["rgn2"]()
            s1["rconv2"]()
        else:
            for p_ in ph:
                for k in ("load", "gn1", "conv1", "gn2", "conv2", "kqv", "attn",
                          "rgn1", "rconv1", "rgn2", "rconv2"):
                    p_[k]()

    nc.finalize()
    return nc


def _pack_conv_w(w):
    """(O, I, 3) -> [P, icb, tap, oc]."""
    w = np.asarray(w, dtype=np.float32)
    o, i, k = w.shape
    return np.ascontiguousarray(w.transpose(1, 2, 0).reshape(CB, P, 3, o).transpose(1, 0, 2, 3))


def _pack_gn(v):
    """(256,) -> [P, CB]"""
    return np.ascontiguousarray(np.asarray(v, dtype=np.float32).reshape(CB, P).T)


def make_in_maps(inp, use_bias):
    """Host-side packing; returns the per-core input maps."""
    gind = np.zeros((P, GPB), np.float32)
    bind = np.zeros((CB, P, P), np.float32)
    for cc in range(P):
        gind[cc, cc // 8] = 0.125
        for cb in range(CB):
            bind[cb, cb * 32 + cc // 8, cc] = 1.0
    shared = {
        "wkqvt": np.ascontiguousarray(
            inp["lin_w"][:, :, 0].T.reshape(CB, P, 3 * C).transpose(1, 0, 2)
        ),
        "gind": gind,
        "bind": bind,
        "ones8": np.ones((P, CB, 16), ml_dtypes.float8_e4m3),
        "onesr": np.ones((1, P), np.float32),
    }
    for rb in ("r1", "r2"):
        shared[f"{rb}_w1t"] = _pack_conv_w(inp[f"{rb}_c1_w"])
        shared[f"{rb}_w2t"] = _pack_conv_w(inp[f"{rb}_c2_w"])
        for ln in (1, 2):
            shared[f"{rb}_gn{ln}_ws"] = _pack_gn(inp[f"{rb}_gn{ln}_w"])
            shared[f"{rb}_gn{ln}_bs"] = _pack_gn(inp[f"{rb}_gn{ln}_b"])
    if "c2b_r1" in use_bias:
        shared["r1_c2bs"] = _pack_gn(inp["r1_c2_b"])
    if "c2b_r2" in use_bias:
        shared["r2_c2bs"] = _pack_gn(inp["r2_c2_b"])
    if "linb" in use_bias:
        shared["lin_bs"] = np.ascontiguousarray(inp["lin_b"].reshape(3 * CB, P).T)

    # per-sample conv1 bias vector: t[s] + c1_b per res block -> [P, CB, 2]
    tfull = inp["t"][:, :, 0]  # (B, C)
    nb = inp["x"].shape[0]
    tv = np.empty((nb, P, CB, 2), np.float32)
    for rbi, rb in enumerate(("r1", "r2")):
        v = tfull + inp[f"{rb}_c1_b"][None, :]
        tv[:, :, :, rbi] = v.reshape(nb, CB, P).transpose(0, 2, 1)

    in_maps = []
    for c in range(NCORES):
        sl = slice(S * c, S * (c + 1))
        m = dict(shared)
        m["x"] = inp["x"][sl]
        m["tv"] = np.ascontiguousarray(tv[sl])
        in_maps.append(m)
    return in_maps


_CACHE = {}


def kernel(**inputs):
    inp = {k: np.ascontiguousarray(np.asarray(v, dtype=np.float32)) for k, v in inputs.items()}

    use_bias = []
    if np.any(inp["r1_c2_b"]):
        use_bias.append("c2b_r1")
    if np.any(inp["r2_c2_b"]):
        use_bias.append("c2b_r2")
    if np.any(inp["lin_b"]):
        use_bias.append("linb")
    use_bias = tuple(use_bias)

    if ("nc", use_bias) not in _CACHE:
        _CACHE[("nc", use_bias)] = build_program(S, use_bias)
    nc = _CACHE[("nc", use_bias)]

    in_maps = make_in_maps(inp, use_bias)
    res = _bu.run_bass_kernel_spmd(nc, in_maps, core_ids=list(range(NCORES)))
    out = np.concatenate([res.results[c]["out"] for c in range(NCORES)], axis=0)
    return out.astype(np.float32)


# revision 38
# speedup vs baseline: 1.0155x; 1.0155x over previous
"""Trainium2 Bass kernel for nn_MidAttnBlock (res-block -> full LxL attention -> res-block).

Contract: kernel(**inputs) takes the FULL inputs of reference.setup_inputs()
(x: (16,256,2048) f32, t: (16,256,1) f32, plus conv/groupnorm/linear params)
and returns the FULL (16,256,2048) f32 output.  Data-parallel over batch on
8 NeuronCores, 2 samples per core; each core runs an identical Bass program.

Convs run in float32r (full-rate PE).  The kqv projection and the attention
(scores, softmax weights, A@V) run in fp8e4 with DoubleRow matmuls (two
128-deep contraction tiles per PE pass), which roughly halves the PE slots
for the attention block at ~6e-3 end-to-end relative error (threshold 2e-2).
exp is computed as exp(s/16 - 4) so the softmax weights fit fp8e4's range;
the shift cancels in the normalization.

The two samples on each core are issued interleaved
(s0:r1,kqv | s1:r1 | s0:attn | s1:kqv | s0:r2 | s1:attn | s1:r2) so the
groupnorm stat chains and softmax tails of one sample overlap the other
sample's matmuls.

Self-contained: all shapes/sharding hardcoded.
"""

import json as _json

import ml_dtypes
import numpy as np

import concourse.bass as bass
import concourse.bass2jax as _b2j
import concourse.bass_utils as _bu
import concourse.tile as tile
from concourse import mybir
from concourse.vector_clock import ScopedClock, VectorClock


def _split_bir_waits(bir_json):
    """The walrus_driver in this container encodes at most ONE sync-wait per
    instruction (and none on Drain).  Tile's sem assigner attaches several.
    Rewrite the BIR: excess waits move to single-wait NoOps inserted directly
    before the instruction on the same engine."""
    m = _json.loads(bir_json)
    ctr = 0
    for fn in m.get("functions", []):
        for bb in fn.get("blocks", []):
            out = []
            for ins in bb.get("instructions", []):
                si = ins.get("sync_info")
                waits = (si or {}).get("on_wait") or []
                keep = 0 if ins.get("opcode") == "Drain" else 1
                if len(waits) > keep:
                    nmove = len(waits) - keep
                    for w in waits[:nmove]:
                        ctr += 1
                        out.append({
                            "debug": ins.get("debug", 0),
                            "engine": ins["engine"],
                            "ins": [],
                            "name": f"{ins['name']}-wsp{ctr}",
                            "opcode": "NoOp",
                            "outs": [],
                            "sync_info": {"on_update": [], "on_wait": [w]},
                        })
                    si["on_wait"] = waits[nmove:]
                out.append(ins)
            bb["instructions"] = out
    return _json.dumps(m).encode()


_orig_compile_bir_kernel = _bu.compile_bir_kernel


def _compile_bir_splitwaits(bir_json, tmpdir, neff_name="file.neff"):
    return _orig_compile_bir_kernel(_split_bir_waits(bir_json), tmpdir, neff_name)


if getattr(_bu.compile_bir_kernel, "__name__", "") != "_compile_bir_splitwaits":
    _bu.compile_bir_kernel = _compile_bir_splitwaits
    _b2j.compile_bir_kernel = _compile_bir_splitwaits


F32 = mybir.dt.float32
F32R = mybir.dt.float32r
FP8 = mybir.dt.float8e4
AF = mybir.ActivationFunctionType
OP = mybir.AluOpType
DR = mybir.MatmulPerfMode.DoubleRow

P = 128          # partitions
C = 256          # channels
CB = 2           # channel blocks of 128
L = 2048         # sequence length
LS = 512         # l-slice (matmul moving dim)
NL = L // LS     # 4 slices
KB = L // P      # 16 k-blocks for attention
NP = KB // 2     # 8 k-block pairs (DoubleRow)
GPB = 16         # groups per channel-block (32 groups, 8 ch each)
EPS = 1e-5
S = 2            # samples per core
NCORES = 8
SCALE = 1.0 / 16.0   # 1/sqrt(C)
ESHIFT = -4.0        # exp(s*SCALE + ESHIFT): keeps softmax weights in fp8e4 range


class _TileContextPatched(tile.TileContext):
    """TileContext whose kernel-tail drain carries no sem waits (the container
    walrus rejects waits on Drain); one SP NOP per proc carries them instead."""

    def _drain_and_barrier(self, tick_clock, wait_clock):
        gc = tick_clock.global_clock
        n = len(gc)
        for p in range(n):
            v = gc[p]
            if v > 0:
                vec = [0] * n
                vec[p] = v
                nop = self.nc.sync.nop()
                wait_clock.add_sem_waits(nop.ins, ScopedClock({None: VectorClock(vec)}))
        self.nc.sync.drain()
        self.nc.all_engine_barrier()
        assert self.sems is not None
        popped = self.nc._tile_sem_poison_stack.pop()
        assert popped is self._sem_poison
        self.nc.clear_and_free_semaphores(list(self.sems.allocated().values()))
        self.nc.all_engine_barrier()


def _f(ap):
    """Read an f32r tile as plain f32 (same bits) for VectorE/ScalarE inputs."""
    return ap.bitcast(F32)


def build_program(samples=S, use_bias=()):
    """Build the per-core Bass program (identical on all cores).

    use_bias: subset of {"c2b_r1", "c2b_r2", "linb"} enabling extra adds for
    biases that setup_inputs() keeps at zero.
    """
    nc = bass.Bass()

    # ---- DRAM I/O (per core) ----
    x_d = nc.dram_tensor("x", (samples, C, L), F32R, kind="ExternalInput")
    # t + conv1 bias, host-packed [samples, P, CB, 2(resblock)]
    t_d = nc.dram_tensor("tv", (samples, P, CB, 2), F32, kind="ExternalInput")
    w_conv = {}
    for rb in ("r1", "r2"):
        # host-packed [P(ic within block), icb, tap, oc]
        w_conv[rb, 1] = nc.dram_tensor(f"{rb}_w1t", (P, CB, 3, C), F32R, kind="ExternalInput")
        w_conv[rb, 2] = nc.dram_tensor(f"{rb}_w2t", (P, CB, 3, C), F32R, kind="ExternalInput")
    wkqv_d = nc.dram_tensor("wkqvt", (P, CB, 3 * C), F32R, kind="ExternalInput")
    gnw_d = {}
    for rb in ("r1", "r2"):
        for ln in (1, 2):
            gnw_d[rb, ln, "w"] = nc.dram_tensor(f"{rb}_gn{ln}_ws", (P, CB), F32, kind="ExternalInput")
            gnw_d[rb, ln, "b"] = nc.dram_tensor(f"{rb}_gn{ln}_bs", (P, CB), F32, kind="ExternalInput")
    c2b_d = {}
    if "c2b_r1" in use_bias:
        c2b_d["r1"] = nc.dram_tensor("r1_c2bs", (P, CB), F32, kind="ExternalInput")
    if "c2b_r2" in use_bias:
        c2b_d["r2"] = nc.dram_tensor("r2_c2bs", (P, CB), F32, kind="ExternalInput")
    linb_d = None
    if "linb" in use_bias:
        linb_d = nc.dram_tensor("lin_bs", (P, 3 * CB), F32, kind="ExternalInput")
    gind_d = nc.dram_tensor("gind", (P, GPB), F32R, kind="ExternalInput")  # 1/8 group indicator
    bind_d = nc.dram_tensor("bind", (CB, P, P), F32R, kind="ExternalInput")    # group->channel broadcast
    ones8_d = nc.dram_tensor("ones8", (P, CB, 16), FP8, kind="ExternalInput")
    onesr_d = nc.dram_tensor("onesr", (1, P), F32R, kind="ExternalInput")
    out_d = nc.dram_tensor("out", (samples, C, L), F32, kind="ExternalOutput")

    with _TileContextPatched(nc) as tc, \
         tc.tile_pool(name="consts", bufs=1) as consts, \
         tc.tile_pool(name="padp", bufs=4) as padp, \
         tc.tile_pool(name="actp", bufs=2) as actp, \
         tc.tile_pool(name="seqp", bufs=2) as seqp, \
         tc.tile_pool(name="vtp", bufs=1) as vtp, \
         tc.tile_pool(name="expp", bufs=3) as expp, \
         tc.tile_pool(name="outp", bufs=2) as outp, \
         tc.tile_pool(name="rdbp", bufs=2) as rdbp, \
         tc.tile_pool(name="rdsp", bufs=2) as rdsp, \
         tc.tile_pool(name="small", bufs=4) as small, \
         tc.tile_pool(name="t2p", bufs=2) as t2p, \
         tc.tile_pool(name="pacc", bufs=3, space="PSUM") as pacc, \
         tc.tile_pool(name="psc", bufs=2, space="PSUM") as psc, \
         tc.tile_pool(name="paux", bufs=1, space="PSUM") as paux:

        # ---- persistent constants / weights in SBUF ----
        # All const loads go on the ScalarE DMA queue (sync/gpsimd queues are
        # reserved for the x loads that gate the first groupnorm), ordered so
        # the tensors gating the head of the pipeline land first.
        gind_sb = consts.tile([P, GPB], F32R, tag="gind", name="gind")
        nc.scalar.dma_start(gind_sb[:], gind_d[:])
        bind_sb = consts.tile([P, CB, P], F32R, tag="bind", name="bind")
        nc.scalar.dma_start(bind_sb[:], bind_d.rearrange("cb p c -> p cb c"))
        gnp_sb = {}
        w1_sb = {}
        w2_sb = {}
        for rb in ("r1", "r2"):
            for ln in (1, 2):
                for wb in ("w", "b"):
                    tl = consts.tile([P, CB], F32, tag=f"gn_{rb}{ln}{wb}", name=f"gn_{rb}{ln}{wb}")
                    nc.scalar.dma_start(tl[:], gnw_d[rb, ln, wb][:])
                    gnp_sb[rb, ln, wb] = tl
            w1_sb[rb] = consts.tile([P, CB, 3, C], F32R, tag=f"w1_{rb}", name=f"w1_{rb}")
            nc.scalar.dma_start(w1_sb[rb][:], w_conv[rb, 1][:])
            w2_sb[rb] = consts.tile([P, CB, 3, C], F32R, tag=f"w2_{rb}", name=f"w2_{rb}")
            nc.scalar.dma_start(w2_sb[rb][:], w_conv[rb, 2][:])
        wkqv_sb = consts.tile([P, CB, 3 * C], F32R, tag="wkqv", name="wkqv")
        nc.scalar.dma_start(wkqv_sb[:], wkqv_d[:])
        c2b_sb = {}
        for rb, d in c2b_d.items():
            c2b_sb[rb] = consts.tile([P, CB], F32, tag=f"c2b_{rb}", name=f"c2b_{rb}")
            nc.scalar.dma_start(c2b_sb[rb][:], d[:])
        linb_sb = None
        if linb_d is not None:
            linb_sb = consts.tile([P, 3 * CB], F32, tag="linb", name="linb")
            nc.scalar.dma_start(linb_sb[:], linb_d[:])
        ones8_sb = consts.tile([P, CB, 16], FP8, tag="ones8", name="ones8")
        nc.scalar.dma_start(ones8_sb[:], ones8_d[:])
        onesr_sb = consts.tile([1, P], F32R, tag="onesr", name="onesr")
        nc.scalar.dma_start(onesr_sb[:], onesr_d[:])
        eps_sb = consts.tile([P, 1], F32, tag="eps", name="eps")
        nc.vector.memset(eps_sb[:], EPS)
        shift_sb = consts.tile([P, 1], F32, tag="shift", name="shift")
        nc.vector.memset(shift_sb[:], ESHIFT)
        zero2 = consts.tile([P, 2], F32, tag="zero2", name="zero2")
        nc.vector.memset(zero2[:], 0.0)

        def alloc_padded(tag, pool):
            """[P, L+2] f32r tile per channel block; data cols [1, L+1), zero edges."""
            ts = []
            for cb in range(CB):
                tl = pool.tile([P, L + 2], F32R, tag=f"{tag}{cb}", name=f"{tag}{cb}")
                nc.vector.tensor_copy(out=tl[:, 0:1], in_=zero2[:, 0:1])
                nc.vector.tensor_copy(out=tl[:, L + 1 : L + 2], in_=zero2[:, 0:1])
                ts.append(tl)
            return ts

        def gn_relu(src, dst, rb, ln):
            """dst = relu(groupnorm(src) * w + b); src/dst are padded f32r pairs.

            One merged nonlinear chain over all 32 groups, then per-block
            broadcast and NL-chunked scalar-engine applies."""
            gp = paux.tile([P, LS], F32, tag="aux", name="gbc")
            for cb in range(CB):
                stats = small.tile([P, NL, 6], F32, tag="stats", name="stats")
                for i in range(NL):
                    nc.vector.bn_stats(out=stats[:, i, :], in_=_f(src[cb][:, 1 + i * LS : 1 + (i + 1) * LS]))
                mv = small.tile([P, 2], F32, tag="mv", name="mv")
                nc.vector.bn_aggr(out=mv[:], in_=stats[:])
                # tmp = [mean_c, E[x^2]_c]  (f32r: feeds the aggregation matmul)
                tmp = small.tile([P, 2], F32R, tag="tmp", name="tmp")
                nc.vector.tensor_copy(out=tmp[:, 0:1], in_=mv[:, 0:1])
                nc.vector.tensor_tensor(out=tmp[:, 1:2], in0=mv[:, 0:1], in1=mv[:, 0:1], op=OP.mult)
                nc.vector.tensor_tensor(out=tmp[:, 1:2], in0=_f(tmp[:, 1:2]), in1=mv[:, 1:2], op=OP.add)
                nc.tensor.matmul(gp[:GPB, 2 * cb : 2 * cb + 2], gind_sb[:], tmp[:], start=True, stop=True)
            # merged group stats; block-cb groups live at partition offset 32*cb
            NG = 32 * CB
            gs = small.tile([NG, 2], F32, tag="gs", name="gs")
            nc.vector.tensor_copy(out=gs[:], in_=zero2[:NG])
            for cb in range(CB):
                nc.vector.tensor_copy(out=gs[cb * 32 : cb * 32 + GPB, :], in_=gp[:GPB, 2 * cb : 2 * cb + 2])
            var = small.tile([NG, 1], F32, tag="var", name="var")
            nc.vector.tensor_tensor(out=var[:], in0=gs[:, 0:1], in1=gs[:, 0:1], op=OP.mult)
            nc.vector.tensor_tensor(out=var[:], in0=gs[:, 1:2], in1=var[:], op=OP.subtract)
            nc.scalar.activation(out=var[:], in_=var[:], func=AF.Ln, bias=eps_sb[:NG])
            rstd = small.tile([NG, 1], F32, tag="rstd", name="rstd")
            nc.scalar.activation(out=rstd[:], in_=var[:], func=AF.Exp, scale=-0.5)
            # pack [rstd_g, -m_g], zero-extended to 128 partitions
            gpk = small.tile([P, 2], F32R, tag="gpk", name="gpk")
            nc.vector.tensor_copy(out=gpk[:], in_=zero2[:])
            nc.vector.tensor_copy(out=gpk[:NG, 0:1], in_=rstd[:])
            nc.vector.tensor_scalar_mul(gpk[:NG, 1:2], gs[:, 0:1], -1.0)
            sbs = []
            for cb in range(CB):
                # broadcast to channels: bc[c, :] = [rstd_g(c), -m_g(c)]
                nc.tensor.matmul(gp[:, 4 + 2 * cb : 6 + 2 * cb], bind_sb[:, cb, :], gpk[:], start=True, stop=True)
                sb = small.tile([P, 2], F32, tag="sb", name="sb")
                # s = rstd*w ; b2 = b - m*s
                nc.vector.tensor_scalar_mul(sb[:, 0:1], gp[:, 4 + 2 * cb : 5 + 2 * cb], gnp_sb[rb, ln, "w"][:, cb : cb + 1])
                nc.vector.tensor_tensor(out=sb[:, 1:2], in0=gp[:, 5 + 2 * cb : 6 + 2 * cb], in1=sb[:, 0:1], op=OP.mult)
                nc.vector.tensor_scalar_add(sb[:, 1:2], sb[:, 1:2], gnp_sb[rb, ln, "b"][:, cb : cb + 1])
                sbs.append(sb)
            # apply + relu on ScalarE in NL chunks, cb-interleaved so the
            # first conv group (which reads both cb blocks) unblocks earliest
            for i in range(NL):
                for cb in range(CB):
                    nc.scalar.activation(
                        out=dst[cb][:, 1 + i * LS : 1 + (i + 1) * LS],
                        in_=_f(src[cb][:, 1 + i * LS : 1 + (i + 1) * LS]),
                        func=AF.Relu,
                        bias=sbs[cb][:, 1:2],
                        scale=sbs[cb][:, 0:1],
                    )

        def conv3(src, wt, consume):
            """3-tap conv over padded f32r src; consume(ocb, ls, psum_tile)."""
            for ls in range(NL):
                for ocb in range(CB):
                    ps = pacc.tile([P, LS], F32, tag="acc", name="acc")
                    k = 0
                    for icb in range(CB):
                        for tap in range(3):
                            nc.tensor.matmul(
                                ps[:],
                                wt[:, icb, tap, ocb * P : (ocb + 1) * P],
                                src[icb][:, ls * LS + tap : ls * LS + tap + LS],
                                start=(k == 0),
                                stop=(k == 5),
                            )
                            k += 1
                    consume(ocb, ls, ps)

        # ------- per-sample state + fine-grained stage closures -------
        def make_sample(s):
            st = {}

            def load():
                with nc.named_scope(f"s{s}_load"):
                    st["xp"] = alloc_padded("pad", padp)
                    for cb in range(CB):
                        for i in range(NL):
                            # split the load across two DMA queues
                            eng = nc.sync if (i % 2 == 0) else nc.gpsimd
                            eng.dma_start(
                                st["xp"][cb][:, 1 + i * LS : 1 + (i + 1) * LS],
                                x_d[s, cb * P : (cb + 1) * P, i * LS : (i + 1) * LS],
                            )
                    st["t2"] = t2p.tile([P, CB, 2], F32, tag="t2", name="t2")
                    nc.sync.dma_start(st["t2"][:], t_d[s])

            def gn1(rb, srckey, dstkey):
                def f():
                    a = alloc_padded("act", actp)
                    st[dstkey] = a
                    with nc.named_scope(f"s{s}_{rb}_gn1"):
                        gn_relu(st[srckey], a, rb, 1)
                return f

            def conv1(rb, rbi, akey, hkey):
                def f():
                    h = alloc_padded("pad", padp)
                    st[hkey] = h
                    t2 = st["t2"]
                    with nc.named_scope(f"s{s}_{rb}_conv1"):
                        def eat1(ocb, ls, ps):
                            nc.vector.tensor_scalar_add(
                                h[ocb][:, 1 + ls * LS : 1 + (ls + 1) * LS], ps[:],
                                t2[:, ocb, rbi : rbi + 1],
                            )
                        conv3(st[akey], w1_sb[rb], eat1)
                return f

            def gn2(rb, hkey, dstkey):
                def f():
                    a2 = alloc_padded("act", actp)
                    st[dstkey] = a2
                    with nc.named_scope(f"s{s}_{rb}_gn2"):
                        gn_relu(st[hkey], a2, rb, 2)
                return f

            def conv2(rb, srckey, a2key, final):
                def f():
                    src = st[srckey]
                    res = None
                    if not final:
                        res = [seqp.tile([P, L], F32R, tag=f"res{cb}", name=f"res{cb}") for cb in range(CB)]
                        st["x1"] = res
                    with nc.named_scope(f"s{s}_{rb}_conv2"):
                        def eat2(ocb, ls, ps):
                            if rb in c2b_sb:
                                nc.vector.tensor_scalar_add(ps[:], ps[:], c2b_sb[rb][:, ocb : ocb + 1])
                            resid = _f(src[ocb][:, 1 + ls * LS : 1 + (ls + 1) * LS])
                            if final:
                                ot = outp.tile([P, LS], F32, tag="out", name="ot")
                                nc.vector.tensor_tensor(out=ot[:], in0=ps[:], in1=resid, op=OP.add)
                                eng = nc.sync if (ls % 2 == 0) else nc.gpsimd
                                eng.dma_start(
                                    out_d[s, ocb * P : (ocb + 1) * P, ls * LS : (ls + 1) * LS], ot[:]
                                )
                            else:
                                nc.vector.tensor_tensor(
                                    out=res[ocb][:, ls * LS : (ls + 1) * LS],
                                    in0=ps[:], in1=resid, op=OP.add,
                                )
                        conv3(st[a2key], w2_sb[rb], eat2)
                return f

            def kqv():
                x1 = st["x1"]
                kt = seqp.tile([P, CB, L], FP8, tag="kt", name="kt")
                qt = seqp.tile([P, CB, L], FP8, tag="qt", name="qt")
                vt = vtp.tile([P, NP, 2, C], FP8, tag="vt", name="vt")
                st["kt"], st["qt"], st["vt"] = kt, qt, vt
                with nc.named_scope(f"s{s}_kqv"):
                    cp = 0
                    for j, dst in ((0, kt), (1, qt)):
                        for ocb in range(CB):
                            off = j * C + ocb * P
                            for ls in range(NL):
                                ps = pacc.tile([P, LS], F32, tag="acc", name="acc")
                                for icb in range(CB):
                                    nc.tensor.matmul(
                                        ps[:],
                                        wkqv_sb[:, icb, off : off + P],
                                        x1[icb][:, ls * LS : (ls + 1) * LS],
                                        start=(icb == 0),
                                        stop=(icb == 1),
                                    )
                                dsl = dst[:, ocb, ls * LS : (ls + 1) * LS]
                                cp += 1
                                if linb_sb is not None:
                                    nc.vector.tensor_scalar_add(
                                        dsl, ps[:], linb_sb[:, j * CB + ocb : j * CB + ocb + 1]
                                    )
                                else:
                                    nc.vector.tensor_copy(out=dsl, in_=ps[:])
                    # vT[l, c] (l on partitions) for the attention output matmul
                    for lb in range(KB):
                        ps = pacc.tile([P, LS], F32, tag="acc", name="acc")
                        for icb in range(CB):
                            nc.tensor.matmul(
                                ps[:, :C],
                                x1[icb][:, lb * P : (lb + 1) * P],
                                wkqv_sb[:, icb, 2 * C : 3 * C],
                                start=(icb == 0),
                                stop=(icb == 1),
                            )
                        # v bias (if any) is added to av after softmax: sum(a)=1
                        nc.vector.tensor_copy(out=vt[:, lb // 2, lb % 2, :], in_=ps[:, :C])

            def attn():
                kt, qt, vt = st["kt"], st["qt"], st["vt"]
                av = alloc_padded("pad", padp)
                st["av"] = av
                for qs in range(NL):
                    with nc.named_scope(f"s{s}_attn{qs}"):
                        dn = paux.tile([P, LS], F32, tag="aux", name="dn")
                        psav = [pacc.tile([P, LS], F32, tag="acc", name="psav") for _ in range(CB)]
                        for p in range(NP):
                            ex = expp.tile([P, 2, LS], FP8, tag="exp", name="exp")
                            sc = psc.tile([P, 2, LS], F32, tag="sc", name="sc")
                            for i in range(2):
                                kbg = 2 * p + i
                                nc.tensor.matmul(
                                    sc[:, i, :],
                                    kt[:, :, kbg * P : (kbg + 1) * P],
                                    qt[:, :, qs * LS : (qs + 1) * LS],
                                    start=True, stop=True, perf_mode=DR,
                                )
                            # one 1024-col exp per k-block pair
                            nc.scalar.activation(
                                out=ex[:, :, :], in_=sc[:, :, :], func=AF.Exp,
                                bias=shift_sb[:], scale=SCALE,
                            )
                            nc.tensor.matmul(
                                dn[0:1, :], ones8_sb[:, :, 0:1], ex[:],
                                start=(p == 0), stop=(p == NP - 1), perf_mode=DR,
                            )
                            for cb in range(CB):
                                nc.tensor.matmul(
                                    psav[cb][:],
                                    vt[:, p, :, cb * P : (cb + 1) * P],
                                    ex[:],
                                    start=(p == 0),
                                    stop=(p == NP - 1),
                                    perf_mode=DR,
                                )
                        lnd = rdsp.tile([1, LS], F32, tag="lnd", name="lnd")
                        nc.scalar.activation(out=lnd[:], in_=dn[0:1, :], func=AF.Ln)
                        rd = rdsp.tile([1, LS], F32R, tag="rd", name="rd")
                        nc.scalar.activation(out=rd[:], in_=lnd[:], func=AF.Exp, scale=-1.0)
                        # broadcast 1/denom across partitions via K=1 ones-matmul
                        rbt = psc.tile([P, 2, LS], F32, tag="sc", name="rb_ps")
                        nc.tensor.matmul(rbt[:, 0, :], onesr_sb[:], rd[:], start=True, stop=True)
                        rdb = rdbp.tile([P, LS], F32, tag="rdbs", name="rdb")
                        nc.scalar.activation(out=rdb[:], in_=rbt[:, 0, :], func=AF.Copy)
                        for cb in range(CB):
                            avs = av[cb][:, 1 + qs * LS : 1 + (qs + 1) * LS]
                            nc.vector.tensor_tensor(out=avs, in0=psav[cb][:], in1=rdb[:], op=OP.mult)
                            if linb_sb is not None:
                                nc.vector.tensor_scalar_add(
                                    avs, _f(avs), linb_sb[:, 2 * CB + cb : 2 * CB + cb + 1]
                                )

            return {
                "load": load,
                "gn1": gn1("r1", "xp", "a"),
                "conv1": conv1("r1", 0, "a", "h"),
                "gn2": gn2("r1", "h", "a2"),
                "conv2": conv2("r1", "xp", "a2", final=False),
                "kqv": kqv,
                "attn": attn,
                "rgn1": gn1("r2", "av", "ra"),
                "rconv1": conv1("r2", 1, "ra", "rh"),
                "rgn2": gn2("r2", "rh", "ra2"),
                "rconv2": conv2("r2", "av", "ra2", final=True),
            }

        ph = [make_sample(s) for s in range(samples)]
        if samples == 2:
            s0, s1 = ph
            # interleave the two samples so every groupnorm stat chain and
            # softmax tail overlaps the other sample's matmuls
            s0["load"](); s1["load"]()
            s0["gn1"](); s1["gn1"]()
            s0["conv1"](); s0["gn2"]()
            s1["conv1"](); s1["gn2"]()
            s0["conv2"]()
            s1["conv2"]()
            s0["kqv"](); s0["attn"]()
            s1["kqv"]()
            s0["rgn1"]()
            s1["attn"]()
            s0["rconv1"]()
            s1["rgn1"]()
            s0["rgn2"]()
            s1["rconv1"]()
            s0["rconv2"]()
            s1["rgn2"]()
            s1["rconv2"]()
        else:
            for p_ in ph:
                for k in ("load", "gn1", "conv1", "gn2", "conv2", "kqv", "attn",
                          "rgn1", "rconv1", "rgn2", "rconv2"):
                    p_[k]()

    nc.finalize()
    return nc


def _pack_conv_w(w):
    """(O, I, 3) -> [P, icb, tap, oc]."""
    w = np.asarray(w, dtype=np.float32)
    o, i, k = w.shape
    return np.ascontiguousarray(w.transpose(1, 2, 0).reshape(CB, P, 3, o).transpose(1, 0, 2, 3))


def _pack_gn(v):
    """(256,) -> [P, CB]"""
    return np.ascontiguousarray(np.asarray(v, dtype=np.float32).reshape(CB, P).T)


def make_in_maps(inp, use_bias):
    """Host-side packing; returns the per-core input maps."""
    gind = np.zeros((P, GPB), np.float32)
    bind = np.zeros((CB, P, P), np.float32)
    for cc in range(P):
        gind[cc, cc // 8] = 0.125
        for cb in range(CB):
            bind[cb, cb * 32 + cc // 8, cc] = 1.0
    shared = {
        "wkqvt": np.ascontiguousarray(
            inp["lin_w"][:, :, 0].T.reshape(CB, P, 3 * C).transpose(1, 0, 2)
        ),
        "gind": gind,
        "bind": bind,
        "ones8": np.ones((P, CB, 16), ml_dtypes.float8_e4m3),
        "onesr": np.ones((1, P), np.float32),
    }
    for rb in ("r1", "r2"):
        shared[f"{rb}_w1t"] = _pack_conv_w(inp[f"{rb}_c1_w"])
        shared[f"{rb}_w2t"] = _pack_conv_w(inp[f"{rb}_c2_w"])
        for ln in (1, 2):
            shared[f"{rb}_gn{ln}_ws"] = _pack_gn(inp[f"{rb}_gn{ln}_w"])
            shared[f"{rb}_gn{ln}_bs"] = _pack_gn(inp[f"{rb}_gn{ln}_b"])
    if "c2b_r1" in use_bias:
        shared["r1_c2bs"] = _pack_gn(inp["r1_c2_b"])
    if "c2b_r2" in use_bias:
        shared["r2_c2bs"] = _pack_gn(inp["r2_c2_b"])
    if "linb" in use_bias:
        shared["lin_bs"] = np.ascontiguousarray(inp["lin_b"].reshape(3 * CB, P).T)

    # per-sample conv1 bias vector: t[s] + c1_b per res block -> [P, CB, 2]
    tfull = inp["t"][:, :, 0]  # (B, C)
    nb = inp["x"].shape[0]
    tv = np.empty((nb, P, CB, 2), np.float32)
    for rbi, rb in enumerate(("r1", "r2")):
        v = tfull + inp[f"{rb}_c1_b"][None, :]
        tv[:, :, :, rbi] = v.reshape(nb, CB, P).transpose(0, 2, 1)

    in_maps = []
    for c in range(NCORES):
        sl = slice(S * c, S * (c + 1))
        m = dict(shared)
        m["x"] = inp["x"][sl]
        m["tv"] = np.ascontiguousarray(tv[sl])
        in_maps.append(m)
    return in_maps


_CACHE = {}


def kernel(**inputs):
    inp = {k: np.ascontiguousarray(np.asarray(v, dtype=np.float32)) for k, v in inputs.items()}

    use_bias = []
    if np.any(inp["r1_c2_b"]):
        use_bias.append("c2b_r1")
    if np.any(inp["r2_c2_b"]):
        use_bias.append("c2b_r2")
    if np.any(inp["lin_b"]):
        use_bias.append("linb")
    use_bias = tuple(use_bias)

    if ("nc", use_bias) not in _CACHE:
        _CACHE[("nc", use_bias)] = build_program(S, use_bias)
    nc = _CACHE[("nc", use_bias)]

    in_maps = make_in_maps(inp, use_bias)
    res = _bu.run_bass_kernel_spmd(nc, in_maps, core_ids=list(range(NCORES)))
    out = np.concatenate([res.results[c]["out"] for c in range(NCORES)], axis=0)
    return out.astype(np.float32)
